# revision 18
# baseline (speedup 1.0000x reference)
"""CosineAttention Trainium2 kernel.

reference:
    xn  = x / max(||x_i||, eps)        # row-normalize
    sim = xn @ xn.T                    # [N, N]
    out = sigmoid(sim @ x)             # [N, D]

Key identity (matmul associativity):
    sim @ x = xn @ (xn^T @ x) = xn @ A,   A = xn^T x  [D, D]
which is O(N D^2) instead of O(N^2 D): 8x less compute (N/D = 8).
Further, A = B^T B with B = x / sqrt(||x_i||)  (symmetric PSD form), so each
core only materializes one scaled copy B of its row block, and
    out = sigmoid( diag(1/sqrt(||x_i||)) (B @ A) ).

Distribution (8 cores, 1-D row-parallel):
    Core c owns ROWS = N/8 rows.  It computes the partial A_c = B_c^T B_c
    [D, D] from its block (contraction over rows lies on the partition axis
    -- no transposes needed), AllReduces A, then computes
    out_c = sigmoid(sinv * (B_c @ A)).

Collective pipelining: A is split into two column halves, each AllReduced
in fp16 (1 MB wire each) as soon as its MM-A half finishes; B^T transposes
and MM2 on half 0 overlap the AllReduce of half 1.  MM-A runs in f32r
(full PE rate); MM2 runs in fp16 (also full rate, inputs already rounded
by the fp16 collective).
"""

import sys

if "/opt/trn_rl_repo" not in sys.path:
    sys.path.insert(0, "/opt/trn_rl_repo")

import numpy as np

N, D = 8192, 1024
NCORES = 8
ROWS = N // NCORES  # 1024 rows per core
P = 128
MC = ROWS // P  # 8 row chunks per core
KC = D // P  # 8 k chunks (contraction dim of MM2 / output rows of A)
DB = 512  # moving free dim for both matmuls
ND = D // DB  # 2 d-halves

_prog_cache = {}


def _build_program():
    import concourse.bass as bass
    import concourse.bacc as bacc
    import concourse.mybir as mybir
    import concourse.tile as tile
    from concourse.masks import make_identity

    f32 = mybir.dt.float32
    f32r = mybir.dt.float32r
    f16 = mybir.dt.float16
    AF = mybir.ActivationFunctionType

    nc = bacc.Bacc(
        trn_type="TRN2", target_bir_lowering=False, debug=False, num_devices=NCORES
    )
    xloc_d = nc.dram_tensor("xloc", [ROWS, D], f32, kind="ExternalInput").ap()
    out_d = nc.dram_tensor("out", [ROWS, D], f32, kind="ExternalOutput").ap()

    with tile.TileContext(nc) as tc:
        with (
            tc.tile_pool(name="singles", bufs=1) as singles,
            tc.tile_pool(name="xl", bufs=3) as xl_pool,
            tc.tile_pool(name="scr", bufs=2) as scr_pool,
            tc.tile_pool(name="ast", bufs=4) as ast_pool,
            tc.tile_pool(name="asb", bufs=2) as asb_pool,
            tc.tile_pool(name="outp", bufs=3) as out_pool,
            tc.tile_pool(name="small", bufs=4) as small,
            tc.tile_pool(name="dram", bufs=2, space="DRAM") as dram,
            tc.tile_pool(name="ps_t", bufs=2, space="PSUM") as ps_t,
            tc.tile_pool(name="ps_a", bufs=3, space="PSUM") as ps_a,
            tc.tile_pool(name="ps_o", bufs=3, space="PSUM") as ps_o,
        ):
            # identity for PE transposes; bounce via DVE so matmul readers
            # never wait on gpsimd (walrus sync-wait limit).
            ident_g = singles.tile([P, P], f32)
            make_identity(nc, ident_g)
            ident_r = singles.tile([P, P], f32r)
            nc.vector.tensor_copy(out=ident_r, in_=ident_g)

            B = singles.tile([P, MC, D], f32r)  # x_loc rows scaled by 1/sqrt(nrm)
            BT = singles.tile([P, KC, ROWS], f16)  # B^T (k on partitions)
            sinv = singles.tile([P, MC], f32)  # 1/sqrt(||row||)

            # per d-half bounce buffers for the column-split AllReduce
            a_part = [
                dram.tile([D, DB], f16, tag=f"ap{h}", name=f"a_part{h}")
                for h in range(ND)
            ]
            a_red = [
                dram.tile(
                    [D, DB], f16, tag=f"ar{h}", name=f"a_red{h}", addr_space="Shared"
                )
                for h in range(ND)
            ]

            # ---- load x_loc, compute norms, scale into B ----
            for rc in range(MC):
                xl = xl_pool.tile([P, D], f32, tag="xl")
                # alternate the two HWDGE queues (SP / ACT) to halve load time
                deng = nc.sync if rc % 2 == 0 else nc.scalar
                deng.dma_start(out=xl, in_=xloc_d[rc * P : (rc + 1) * P, :])
                sq = scr_pool.tile([P, D], f32, tag="sq")
                ssq = small.tile([P, 1], f32, tag="ssq")
                nc.scalar.activation(out=sq, in_=xl, func=AF.Square, accum_out=ssq)
                nrm = small.tile([P, 1], f32, tag="nrm")
                nc.scalar.activation(out=nrm, in_=ssq, func=AF.Sqrt)
                inv = small.tile([P, 1], f32, tag="inv")
                nc.vector.reciprocal(inv, nrm)
                nc.scalar.activation(
                    out=sinv[:, rc : rc + 1], in_=inv, func=AF.Sqrt
                )
                nc.vector.tensor_scalar_mul(
                    out=B[:, rc, :], in0=xl, scalar1=sinv[:, rc : rc + 1]
                )

            # ---- MM-A + per-half AllReduce ----
            # A_c[k, d] = sum_r B[r, k] B[r, d]  (r on partitions)
            for dh in range(ND):
                for kc in range(KC):
                    ps = ps_a.tile([P, DB], f32)
                    for rc in range(MC):
                        nc.tensor.matmul(
                            ps,
                            B[:, rc, kc * P : (kc + 1) * P],
                            B[:, rc, dh * DB : (dh + 1) * DB],
                            start=(rc == 0),
                            stop=(rc == MC - 1),
                        )
                    a_st = ast_pool.tile([P, DB], f16, tag="ast")
                    nc.vector.tensor_copy(out=a_st, in_=ps)
                    nc.sync.dma_start(
                        out=a_part[dh][kc * P : (kc + 1) * P, :], in_=a_st
                    )
                nc.gpsimd.collective_compute(
                    "AllReduce",
                    mybir.AluOpType.add,
                    replica_groups=[list(range(NCORES))],
                    ins=[a_part[dh][:].opt()],
                    outs=[a_red[dh][:].opt()],
                )

            # ---- build B^T on PE while the collectives are in flight ----
            for rc in range(MC):
                for kg in range(KC // 4):
                    pst = ps_t.tile([P, 4 * P], f32)
                    for j3 in range(4):
                        kc = kg * 4 + j3
                        nc.tensor.transpose(
                            pst[:, j3 * P : (j3 + 1) * P].bitcast(f32r),
                            B[:, rc, kc * P : (kc + 1) * P],
                            ident_r,
                        )
                    nc.vector.tensor_copy(
                        out=BT[:, kg * 4 : (kg + 1) * 4, rc * P : (rc + 1) * P],
                        in_=pst.rearrange("p (k q) -> p k q", k=4),
                    )

            # ---- MM2: out = sigmoid(sinv * (B @ A)), d-half at a time ----
            for dh in range(ND):
                a_sb = asb_pool.tile([P, KC, DB], f16, tag="asb")
                nc.scalar.dma_start(
                    out=a_sb,
                    in_=a_red[dh][:].rearrange("(kc p) d -> p kc d", p=P),
                )
                for mc in range(MC):
                    ps2 = ps_o.tile([P, DB], f32)
                    for kc in range(KC):
                        nc.tensor.matmul(
                            ps2,
                            BT[:, kc, mc * P : (mc + 1) * P],
                            a_sb[:, kc, :],
                            start=(kc == 0),
                            stop=(kc == KC - 1),
                        )
                    ot = out_pool.tile([P, DB], f32, tag="ot")
                    nc.scalar.activation(
                        out=ot,
                        in_=ps2,
                        func=AF.Sigmoid,
                        scale=sinv[:, mc : mc + 1],
                    )
                    nc.sync.dma_start(
                        out=out_d[mc * P : (mc + 1) * P, dh * DB : (dh + 1) * DB],
                        in_=ot,
                    )

    nc.compile()
    return nc


def get_program():
    if "nc" not in _prog_cache:
        _prog_cache["nc"] = _build_program()
    return _prog_cache["nc"]


def kernel(x: np.ndarray, W: np.ndarray, _collect=None) -> np.ndarray:
    """Full-input / full-output entry point. W is an unused declared param."""
    from concourse.bass_utils import run_bass_kernel_spmd

    nc = get_program()
    x = np.ascontiguousarray(np.asarray(x, dtype=np.float32))
    in_maps = [{"xloc": x[c * ROWS : (c + 1) * ROWS]} for c in range(NCORES)]
    res = run_bass_kernel_spmd(
        nc, in_maps, list(range(NCORES)), trace=bool(_collect is not None)
    )
    if _collect is not None:
        _collect["results"] = res
    return np.concatenate([res.results[c]["out"] for c in range(NCORES)], axis=0)


if __name__ == "__main__":
    get_program()
    print("program built OK")


# revision 19
# speedup vs baseline: 1.0195x; 1.0195x over previous
"""CosineAttention Trainium2 kernel.

reference:
    xn  = x / max(||x_i||, eps)        # row-normalize
    sim = xn @ xn.T                    # [N, N]
    out = sigmoid(sim @ x)             # [N, D]

Key identity (matmul associativity):
    sim @ x = xn @ (xn^T @ x) = xn @ A,   A = xn^T x  [D, D]
which is O(N D^2) instead of O(N^2 D): 8x less compute (N/D = 8).
Further, A = B^T B with B = x / sqrt(||x_i||)  (symmetric PSD form), so each
core only materializes one scaled copy B of its row block, and
    out = sigmoid( diag(1/sqrt(||x_i||)) (B @ A) ).

Distribution (8 cores, 1-D row-parallel):
    Core c owns ROWS = N/8 rows.  It computes the partial A_c = B_c^T B_c
    [D, D] from its block (contraction over rows lies on the partition axis
    -- no transposes needed), AllReduces A, then computes
    out_c = sigmoid(sinv * (B_c @ A)).

Collective pipelining: A is split into two column halves, each AllReduced
in fp16 (1 MB wire each) as soon as its MM-A half finishes; B^T transposes
and MM2 on half 0 overlap the AllReduce of half 1.  MM-A runs in f32r
(full PE rate); MM2 runs in fp16 (also full rate, inputs already rounded
by the fp16 collective).
"""

import sys

if "/opt/trn_rl_repo" not in sys.path:
    sys.path.insert(0, "/opt/trn_rl_repo")

import numpy as np

N, D = 8192, 1024
NCORES = 8
ROWS = N // NCORES  # 1024 rows per core
P = 128
MC = ROWS // P  # 8 row chunks per core
KC = D // P  # 8 k chunks (contraction dim of MM2 / output rows of A)
DB = 512  # max moving free dim for matmuls
ND = 2  # two column groups for the split AllReduce
# asymmetric split: group 0 = cols [0,768) so MM2 on it covers more of the
# second collective's latency; group 1 = cols [768,1024).  Col-blocks keep
# the moving dim at 512/256 (>=256 keeps f32r at full PE rate).
HOFF = [0, 768]  # half start col
HW_ = [768, 256]  # half width
HBLK = [[(0, 512), (512, 256)], [(768, 256)]]  # (abs col, width) blocks

_prog_cache = {}


def _build_program():
    import concourse.bass as bass
    import concourse.bacc as bacc
    import concourse.mybir as mybir
    import concourse.tile as tile
    from concourse.masks import make_identity

    f32 = mybir.dt.float32
    f32r = mybir.dt.float32r
    f16 = mybir.dt.float16
    AF = mybir.ActivationFunctionType

    nc = bacc.Bacc(
        trn_type="TRN2", target_bir_lowering=False, debug=False, num_devices=NCORES
    )
    xloc_d = nc.dram_tensor("xloc", [ROWS, D], f32, kind="ExternalInput").ap()
    out_d = nc.dram_tensor("out", [ROWS, D], f32, kind="ExternalOutput").ap()

    with tile.TileContext(nc) as tc:
        with (
            tc.tile_pool(name="singles", bufs=1) as singles,
            tc.tile_pool(name="xl", bufs=3) as xl_pool,
            tc.tile_pool(name="scr", bufs=2) as scr_pool,
            tc.tile_pool(name="ast", bufs=4) as ast_pool,
            tc.tile_pool(name="asb", bufs=2) as asb_pool,
            tc.tile_pool(name="outp", bufs=3) as out_pool,
            tc.tile_pool(name="small", bufs=4) as small,
            tc.tile_pool(name="dram", bufs=2, space="DRAM") as dram,
            tc.tile_pool(name="ps_t", bufs=2, space="PSUM") as ps_t,
            tc.tile_pool(name="ps_a", bufs=3, space="PSUM") as ps_a,
            tc.tile_pool(name="ps_o", bufs=3, space="PSUM") as ps_o,
        ):
            # identity for PE transposes; bounce via DVE so matmul readers
            # never wait on gpsimd (walrus sync-wait limit).
            ident_g = singles.tile([P, P], f32)
            make_identity(nc, ident_g)
            ident_r = singles.tile([P, P], f32r)
            nc.vector.tensor_copy(out=ident_r, in_=ident_g)

            B = singles.tile([P, MC, D], f32r)  # x_loc rows scaled by 1/sqrt(nrm)
            BT = singles.tile([P, KC, ROWS], f16)  # B^T (k on partitions)
            sinv = singles.tile([P, MC], f32)  # 1/sqrt(||row||)

            # per column-group bounce buffers for the split AllReduce
            a_part = [
                dram.tile([D, HW_[h]], f16, tag=f"ap{h}", name=f"a_part{h}")
                for h in range(ND)
            ]
            a_red = [
                dram.tile(
                    [D, HW_[h]], f16, tag=f"ar{h}", name=f"a_red{h}",
                    addr_space="Shared",
                )
                for h in range(ND)
            ]

            # ---- load x_loc, compute norms, scale into B ----
            for rc in range(MC):
                xl = xl_pool.tile([P, D], f32, tag="xl")
                # alternate the two HWDGE queues (SP / ACT) to halve load time
                deng = nc.sync if rc % 2 == 0 else nc.scalar
                deng.dma_start(out=xl, in_=xloc_d[rc * P : (rc + 1) * P, :])
                sq = scr_pool.tile([P, D], f32, tag="sq")
                ssq = small.tile([P, 1], f32, tag="ssq")
                nc.scalar.activation(out=sq, in_=xl, func=AF.Square, accum_out=ssq)
                nrm = small.tile([P, 1], f32, tag="nrm")
                nc.scalar.activation(out=nrm, in_=ssq, func=AF.Sqrt)
                inv = small.tile([P, 1], f32, tag="inv")
                nc.vector.reciprocal(inv, nrm)
                nc.scalar.activation(
                    out=sinv[:, rc : rc + 1], in_=inv, func=AF.Sqrt
                )
                nc.vector.tensor_scalar_mul(
                    out=B[:, rc, :], in0=xl, scalar1=sinv[:, rc : rc + 1]
                )

            # ---- MM-A + per-group AllReduce ----
            # A_c[k, d] = sum_r B[r, k] B[r, d]  (r on partitions)
            for dh in range(ND):
                for col, bw in HBLK[dh]:
                    for kc in range(KC):
                        ps = ps_a.tile([P, DB], f32)
                        for rc in range(MC):
                            nc.tensor.matmul(
                                ps[:, :bw],
                                B[:, rc, kc * P : (kc + 1) * P],
                                B[:, rc, col : col + bw],
                                start=(rc == 0),
                                stop=(rc == MC - 1),
                            )
                        a_st = ast_pool.tile([P, DB], f16, tag="ast")
                        nc.vector.tensor_copy(out=a_st[:, :bw], in_=ps[:, :bw])
                        nc.sync.dma_start(
                            out=a_part[dh][
                                kc * P : (kc + 1) * P,
                                col - HOFF[dh] : col - HOFF[dh] + bw,
                            ],
                            in_=a_st[:, :bw],
                        )
                nc.gpsimd.collective_compute(
                    "AllReduce",
                    mybir.AluOpType.add,
                    replica_groups=[list(range(NCORES))],
                    ins=[a_part[dh][:].opt()],
                    outs=[a_red[dh][:].opt()],
                )

            # ---- build B^T on PE while the collectives are in flight ----
            for rc in range(MC):
                for kg in range(KC // 4):
                    pst = ps_t.tile([P, 4 * P], f32)
                    for j3 in range(4):
                        kc = kg * 4 + j3
                        nc.tensor.transpose(
                            pst[:, j3 * P : (j3 + 1) * P].bitcast(f32r),
                            B[:, rc, kc * P : (kc + 1) * P],
                            ident_r,
                        )
                    nc.vector.tensor_copy(
                        out=BT[:, kg * 4 : (kg + 1) * 4, rc * P : (rc + 1) * P],
                        in_=pst.rearrange("p (k q) -> p k q", k=4),
                    )

            # ---- MM2: out = sigmoid(sinv * (B @ A)), col-group at a time ----
            for dh in range(ND):
                a_sb = asb_pool.tile([P, KC, HW_[dh]], f16, tag=f"asb{dh}")
                nc.scalar.dma_start(
                    out=a_sb,
                    in_=a_red[dh][:].rearrange("(kc p) d -> p kc d", p=P),
                )
                for col, bw in HBLK[dh]:
                    boff = col - HOFF[dh]
                    for mc in range(MC):
                        ps2 = ps_o.tile([P, DB], f32)
                        for kc in range(KC):
                            nc.tensor.matmul(
                                ps2[:, :bw],
                                BT[:, kc, mc * P : (mc + 1) * P],
                                a_sb[:, kc, boff : boff + bw],
                                start=(kc == 0),
                                stop=(kc == KC - 1),
                            )
                        ot = out_pool.tile([P, DB], f32, tag="ot")
                        nc.scalar.activation(
                            out=ot[:, :bw],
                            in_=ps2[:, :bw],
                            func=AF.Sigmoid,
                            scale=sinv[:, mc : mc + 1],
                        )
                        nc.sync.dma_start(
                            out=out_d[mc * P : (mc + 1) * P, col : col + bw],
                            in_=ot[:, :bw],
                        )

    nc.compile()
    return nc


def get_program():
    if "nc" not in _prog_cache:
        _prog_cache["nc"] = _build_program()
    return _prog_cache["nc"]


def kernel(x: np.ndarray, W: np.ndarray, _collect=None) -> np.ndarray:
    """Full-input / full-output entry point. W is an unused declared param."""
    from concourse.bass_utils import run_bass_kernel_spmd

    nc = get_program()
    x = np.ascontiguousarray(np.asarray(x, dtype=np.float32))
    in_maps = [{"xloc": x[c * ROWS : (c + 1) * ROWS]} for c in range(NCORES)]
    res = run_bass_kernel_spmd(
        nc, in_maps, list(range(NCORES)), trace=bool(_collect is not None)
    )
    if _collect is not None:
        _collect["results"] = res
    return np.concatenate([res.results[c]["out"] for c in range(NCORES)], axis=0)


if __name__ == "__main__":
    get_program()
    print("program built OK")
